# revision 17
# baseline (speedup 1.0000x reference)
"""Trainium2 Bass kernel for a fused GRUCell step.

Math (reference):
    xi = x @ [W_ir W_iz W_in] + [b_ir b_iz b_in]
    hh = h @ [W_hr W_hz W_hn]
    r = sigmoid(xr + hr); z = sigmoid(xz + hz)
    n = tanh(xn + r * (hn + b_hn))
    new_h = (1 - z) * n + z * h

Strategy (v3, HW-measured 281us vs 397us v1 baseline; rel err 1.21e-2):

- 2D shard — batch 4-way x hidden 2-way (8 cores), each core a
  [B_CORE=4096, H_CORE=512] output tile. Weights per core shrink to ~4.5MB
  (vs 12.6MB replicated), eliminating the startup DMA wall that dominated
  v1 (first matmul at t=52us waiting for all replicated weights to land).
- The GEMM runs with the *weights* stationary and 512-wide batch blocks
  moving, so the weight-arrival rate needed at startup is 8x lower than
  batch-stationary. Outputs land transposed ([H-partition, batch-free]);
  h is supplied host-transposed and the final unshard transposes back.
  Per-gate biases become per-partition scalars, fused into ScalarE
  activations for free.
- Precision: z gate and the x-part of n stay fp16 (1 cycle/row on the PE);
  the r gate and the h-part of n run fp8(e4m3) with perf_mode=DoubleRow,
  packing K=256 per matmul — 2x PE throughput for those sweeps. Weights are
  pre-scaled by WS=16 into the e4m3 normal range; the 1/WS rescale folds
  into the sigmoid activation scale (r) / one tensor_scalar_mul (hn).
  Measured end-to-end rel err 1.21e-2 (numpy-simulated and HW-confirmed to
  4 digits) against the 2e-2 gate; all-fp8 (3e-2) and z-fp8 (1.8e-2 alone)
  do not fit. The hn DR sweep ends each chunk so it sits adjacent to the
  next chunk's DR r-sweep — fp16<->fp8 PE mode flips cost ~190ns each and
  are kept to 2 per chunk.
- DMA issue order is hand-scheduled on the sync queue (one FIFO, ~290GB/s
  aggregate across 16 engines, bulk flow starts ~10.5us into the kernel):
  first-chunk weights and the first lhsT block go first, remaining weights
  stream in consumption order during block 0, lhsT blocks prefetch mid-block.
  First matmul issues ~13us in; the matmul stream then runs at the 216ns/MM
  issue floor with ~15us total of residual stalls (block-1 lhsT arrival is
  DMA-bandwidth-bound, plus ~200ns DoubleRow LDWEIGHTS bubbles: a DR
  weight load is 256 columns ~ 213ns, nearly the full 216ns matmul slot).
- The last chunk's epilogue runs in two 256-column pieces to shorten the
  post-matmul tail (the gate chain is ~5us deep on DVE/ScalarE otherwise).
  Keep the phn rescale on the vector engine: moving it to ScalarE
  serializes the epilogue chain and cost +46us when tried.

Accumulation is fp32 in PSUM throughout: pr/pz/pxn/phn each take a full
PSUM bank, double-buffered = all 8 banks.
"""

import os
import sys

import numpy as np

sys.path.insert(0, "/opt/trn_rl_repo")
os.environ.setdefault("MYCRO_LOCAL_CACHE", "1")

import concourse.bass as bass  # noqa: E402
import concourse.mybir as mybir  # noqa: E402
import concourse.tile as tile  # noqa: E402
from concourse import bacc  # noqa: E402
from concourse.bass_utils import run_bass_kernel_spmd  # noqa: E402

N_CORES = 8
B_SHARDS = 4
H_SHARDS = 2
B = 16384
F = 1024  # input feature dim
H = 1024  # hidden dim
K = F + H  # GEMM contraction dim (x features then h features)
P = 128
KO = K // P  # 16 k-chunks of 128
KOX = F // P  # 8 k-chunks belonging to the x part
B_CORE = B // B_SHARDS  # 4096
H_CORE = H // H_SHARDS  # 512
MBLK = 512  # batch columns per moving-operand block (= PSUM bank width)
NBLK = B_CORE // MBLK  # 8
HC_N = H_CORE // P  # 4 hidden-column chunks of 128 (PSUM partition dim)
KO8 = K // 256  # 8 k-chunks of 256 for fp8 DoubleRow (r gate)
KO8X = F // 256  # 4 fp8 k-chunks belonging to the x part
KO8H = H // 256  # 4 fp8 k-chunks for the hn part
WS = 16.0  # fp8 weight scale: lifts w std ~0.031 into e4m3 normal range


def build_gru_program(with_bias: bool) -> bass.Bass:
    """One SPMD program; every core runs it on its own (batch, hidden) tile."""
    fp16 = mybir.dt.float16
    f32 = mybir.dt.float32

    # Bacc (not plain Bass): its compile pipeline splits multi-sem waits into
    # event semaphores — walrus rejects >1 wait on most engine instructions.
    nc = bacc.Bacc()
    # Host-prearranged so every DMA lands contiguously per partition:
    # lhsT[b, p, ko, m] = concat(x,h).T[ko*P+p, b*MBLK+m]
    lhsT = nc.declare_dram_parameter("lhsT", [NBLK, P, KO, MBLK], fp16, isOutput=False)
    fp8 = mybir.dt.float8e4
    # z gate (full K) and xn part (first F rows of n gate) stay fp16
    wz = nc.declare_dram_parameter("wz", [HC_N, P, KO, P], fp16, isOutput=False)
    wxn = nc.declare_dram_parameter("wxn", [HC_N, P, KOX, P], fp16, isOutput=False)
    # r gate (full K) + hn part in fp8 DoubleRow layout, combined:
    # j in [0,KO8): WS*Wcat_r[j*256+i*128+p, hc*P+n]
    # j in [KO8, KO8+KO8H): WS*W_hn[(j-KO8)*256+i*128+p, hc*P+n]
    w8rh = nc.declare_dram_parameter(
        "w8rh", [HC_N, P, KO8 + KO8H, 2, P], fp8, isOutput=False
    )
    # fp8 copy of the moving operand for the r gate, DoubleRow-paired
    lhsT8 = nc.declare_dram_parameter(
        "lhsT8", [NBLK, P, KO8, 2, MBLK], fp8, isOutput=False
    )
    # hT[j, m] = h_shard[m, j]  (host-transposed h slice; fp16 — only feeds
    # the z*h blend, where fp16 rounding of h is ~3e-4 relative)
    hT = nc.declare_dram_parameter("hT", [H_CORE, B_CORE], fp16, isOutput=False)
    if with_bias:
        # bias[p, g, hc] = b_g[hj*H_CORE + hc*P + p]; g: 0=b_ir 1=b_iz 2=b_in 3=b_hn
        biasp = nc.declare_dram_parameter("bias", [P, 4, HC_N], f32, isOutput=False)
    out = nc.declare_dram_parameter("out", [H_CORE, B_CORE], f32, isOutput=True)

    Sigmoid = mybir.ActivationFunctionType.Sigmoid
    Tanh = mybir.ActivationFunctionType.Tanh

    with tile.TileContext(nc) as tc:
        with (
            tc.tile_pool(name="wpool", bufs=1) as wpool,
            tc.tile_pool(name="lpool", bufs=3) as lpool,
            tc.tile_pool(name="hpool", bufs=2) as hpool,
            tc.tile_pool(name="opool", bufs=3) as opool,
            tc.tile_pool(name="epool", bufs=2) as epool,
            tc.tile_pool(name="psum", bufs=2, space="PSUM") as psum,
        ):
            wsb = {}
            w8sb = {}

            def load_wz(hc: int):
                t = wpool.tile([P, KO, P], fp16, tag=f"wz_{hc}")
                nc.sync.dma_start(t[:], wz[hc])
                wsb[("z", hc)] = t

            def load_wxn(hc: int):
                t = wpool.tile([P, KOX, P], fp16, tag=f"wxn_{hc}")
                nc.sync.dma_start(t[:], wxn[hc])
                wsb[("xn", hc)] = t

            def load_w8(hc: int):
                t = wpool.tile([P, KO8 + KO8H, 2, P], fp8, tag=f"w8_{hc}")
                nc.sync.dma_start(t[:], w8rh[hc])
                w8sb[hc] = t

            def load_lt8(b: int):
                t = lpool.tile([P, KO8, 2, MBLK], fp8, tag="lt8")
                half = KO8 // 2
                nc.sync.dma_start(t[:, 0:half, :, :], lhsT8[b, :, 0:half, :, :])
                nc.sync.dma_start(t[:, half:KO8, :, :], lhsT8[b, :, half:KO8, :, :])
                return t

            def load_lt(b: int):
                # two ko-halves so the first matmuls start after 1MB, not 2MB
                t = lpool.tile([P, KO, MBLK], fp16, tag="lt")
                half = KO // 2
                nc.sync.dma_start(t[:, 0:half, :], lhsT[b, :, 0:half, :])
                nc.sync.dma_start(t[:, half:KO, :], lhsT[b, :, half:KO, :])
                return t

            # --- startup-critical DMA order (sync queue is one FIFO) ---
            load_w8(0)
            lt8 = load_lt8(0)
            load_wz(0)
            lt = load_lt(0)
            load_wxn(0)
            bias_sb = None
            if with_bias:
                bias_sb = wpool.tile([P, 4, HC_N], f32, tag="bias_sb")
                nc.sync.dma_start(bias_sb[:], biasp[:])

            ht_tiles = {}

            def load_ht(b: int, hc: int):
                t = hpool.tile([P, MBLK], fp16, tag=f"ht{hc}")
                nc.sync.dma_start(
                    t[:], hT[hc * P : (hc + 1) * P, b * MBLK : (b + 1) * MBLK]
                )
                ht_tiles[(b, hc)] = t

            load_ht(0, 0)

            # PE warmup: ~100 tiny matmuls on a zeroed scratch tile while the
            # first operands stream in (~13us). Keeps the PE HAM activity
            # window busy so it unthrottles from 1.2GHz to 2.4GHz before the
            # real matmul stream starts (saves ~14 cold matmuls at 2x cost).
            warm_w = wpool.tile([P, P], fp16, tag="warm_w")
            nc.vector.memset(warm_w[:], 0.0)
            warm_ps = psum.tile([P, MBLK], f32, tag="pr")
            for _ in range(100):
                nc.tensor.matmul(
                    warm_ps[:, 0:P], warm_w[:], warm_w[:], start=True, stop=True
                )

            lt_next = None
            lt8_next = None
            for b in range(NBLK):
                # snake order: odd blocks walk hc in reverse, so the first
                # chunk of block 1 needs the weights that arrive last (hc=3)
                # just when they land, instead of stalling on them mid-block-0
                hcs = range(HC_N) if b % 2 == 0 else range(HC_N - 1, -1, -1)
                for ci, hc in enumerate(hcs):
                    # block 0 pulls in the remaining weights one chunk ahead
                    # of use (incl. hc=3 before the block-1 lhsT prefetch)
                    if b == 0 and ci < HC_N - 1:
                        load_w8(ci + 1)
                        load_wz(ci + 1)
                        load_wxn(ci + 1)
                        # block 0's h tiles, early: their epilogues must not
                        # lag more than the PSUM double-buffer allows
                        load_ht(0, ci + 1)
                        if ci == 1:
                            load_ht(1, 3)  # block 1 starts at hc=3 (snake)
                    # prefetch next batch block mid-way through this one
                    if ci == 2 and b + 1 < NBLK:
                        lt8_next = load_lt8(b + 1)
                        lt_next = load_lt(b + 1)

                    if (b, hc) in ht_tiles:
                        ht = ht_tiles.pop((b, hc))
                    else:
                        ht = hpool.tile([P, MBLK], fp16, tag=f"ht{hc}")
                        nc.sync.dma_start(
                            ht[:],
                            hT[hc * P : (hc + 1) * P, b * MBLK : (b + 1) * MBLK],
                        )

                    pr = psum.tile([P, MBLK], f32, tag="pr")
                    pz = psum.tile([P, MBLK], f32, tag="pz")
                    pxn = psum.tile([P, MBLK], f32, tag="pxn")
                    phn = psum.tile([P, MBLK], f32, tag="phn")

                    # gate sweeps: stationary = weight chunk, moving = batch
                    # r gate: fp8 DoubleRow, K=256 per matmul, result is WS*(xr+hr)
                    for ko8 in range(KO8):
                        nc.tensor.matmul(
                            pr[:],
                            w8sb[hc][:, ko8, :, :],
                            lt8[:, ko8, :, :],
                            start=(ko8 == 0),
                            stop=(ko8 == KO8 - 1),
                            perf_mode=mybir.MatmulPerfMode.DoubleRow,
                        )
                    for ko in range(KO):
                        nc.tensor.matmul(
                            pz[:],
                            wsb[("z", hc)][:, ko, :],
                            lt[:, ko, :],
                            start=(ko == 0),
                            stop=(ko == KO - 1),
                        )
                    for ko in range(KOX):
                        nc.tensor.matmul(
                            pxn[:],
                            wsb[("xn", hc)][:, ko, :],
                            lt[:, ko, :],
                            start=(ko == 0),
                            stop=(ko == KOX - 1),
                        )
                    # hn part: fp8 DoubleRow (ends the chunk so it sits next
                    # to the following chunk's DR r-sweep — fewer mode flips)
                    for j in range(KO8H):
                        nc.tensor.matmul(
                            phn[:],
                            w8sb[hc][:, KO8 + j, :, :],
                            lt8[:, KO8X + j, :, :],
                            start=(j == 0),
                            stop=(j == KO8H - 1),
                            perf_mode=mybir.MatmulPerfMode.DoubleRow,
                        )

                    sr = epool.tile([P, MBLK], f32, tag="sr")
                    sz = epool.tile([P, MBLK], f32, tag="sz")
                    sn = epool.tile([P, MBLK], f32, tag="sn")
                    tt = epool.tile([P, MBLK], f32, tag="tt")
                    ot = opool.tile([P, MBLK], f32, tag="ot")

                    def epilogue(lo: int, hi: int):
                        s = slice(lo, hi)
                        if with_bias:
                            nc.scalar.activation(
                                sr[:, s],
                                pr[:, s],
                                Sigmoid,
                                bias=bias_sb[:, 0, hc : hc + 1],
                                scale=1.0 / WS,
                            )
                            nc.scalar.activation(
                                sz[:, s],
                                pz[:, s],
                                Sigmoid,
                                bias=bias_sb[:, 1, hc : hc + 1],
                            )
                            nc.vector.tensor_scalar(
                                tt[:, s],
                                phn[:, s],
                                1.0 / WS,
                                bias_sb[:, 3, hc : hc + 1],
                                mybir.AluOpType.mult,
                                mybir.AluOpType.add,
                            )
                            nc.vector.tensor_mul(tt[:, s], sr[:, s], tt[:, s])
                            nc.vector.tensor_add(tt[:, s], tt[:, s], pxn[:, s])
                            nc.scalar.activation(
                                sn[:, s],
                                tt[:, s],
                                Tanh,
                                bias=bias_sb[:, 2, hc : hc + 1],
                            )
                        else:
                            nc.scalar.activation(
                                sr[:, s], pr[:, s], Sigmoid, scale=1.0 / WS
                            )
                            nc.scalar.activation(sz[:, s], pz[:, s], Sigmoid)
                            nc.vector.tensor_scalar_mul(tt[:, s], phn[:, s], 1.0 / WS)
                            nc.vector.tensor_mul(tt[:, s], sr[:, s], tt[:, s])
                            nc.vector.tensor_add(tt[:, s], tt[:, s], pxn[:, s])
                            nc.scalar.activation(sn[:, s], tt[:, s], Tanh)
                        nc.vector.tensor_sub(tt[:, s], ht[:, s], sn[:, s])
                        nc.vector.tensor_mul(tt[:, s], tt[:, s], sz[:, s])
                        nc.vector.tensor_add(ot[:, s], sn[:, s], tt[:, s])
                        nc.sync.dma_start(
                            out[
                                hc * P : (hc + 1) * P,
                                b * MBLK + lo : b * MBLK + hi,
                            ],
                            ot[:, s],
                        )

                    if b == NBLK - 1 and ci == HC_N - 1:
                        # last chunk: pipeline the epilogue in column pieces so
                        # the post-matmul tail is short
                        for lo in range(0, MBLK, 2 * P):
                            epilogue(lo, lo + 2 * P)
                    else:
                        epilogue(0, MBLK)
                if lt_next is not None:
                    lt = lt_next
                    lt8 = lt8_next
                    lt_next = None
    nc.finalize()
    return nc


_PROGRAM_CACHE: dict = {}


def get_program(with_bias: bool) -> bass.Bass:
    if with_bias not in _PROGRAM_CACHE:
        _PROGRAM_CACHE[with_bias] = build_gru_program(with_bias)
    return _PROGRAM_CACHE[with_bias]


def prepare_in_maps(h, x, W_ir, W_iz, W_in, b_ir, b_iz, b_in, W_hr, W_hz, W_hn, b_hn):
    """Host-side shard + layout prep. Returns (in_maps, with_bias)."""
    h = np.ascontiguousarray(np.asarray(h, dtype=np.float32))
    x = np.ascontiguousarray(np.asarray(x, dtype=np.float32))
    assert x.shape == (B, F) and h.shape == (B, H), (x.shape, h.shape)

    import ml_dtypes

    fp8np = ml_dtypes.float8_e4m3
    wcat_z = np.concatenate([W_iz, W_hz], axis=0).astype(np.float16)  # [K, H]
    w_xn = np.asarray(W_in, np.float32).astype(np.float16)  # [F, H]
    wcat_r = np.concatenate([W_ir, W_hr], axis=0).astype(np.float32)  # [K, H]
    w_hn = np.asarray(W_hn, np.float32)  # [H, H]

    br = np.asarray(b_ir, np.float32)
    bz = np.asarray(b_iz, np.float32)
    bn = np.asarray(b_in, np.float32)
    bhn = np.asarray(b_hn, np.float32)
    biases = np.stack([br, bz, bn, bhn])  # [4, H]
    with_bias = bool(np.any(biases != 0.0))

    # per H-shard: weights in the exact SBUF layout
    wz_shards = []
    wxn_shards = []
    w8_shards = []
    bias_shards = []
    for hj in range(H_SHARDS):
        cs = slice(hj * H_CORE, (hj + 1) * H_CORE)
        # [K, H_CORE] -> [KO, P, HC_N, P] -> [HC_N, P, KO, P]
        wzs = wcat_z[:, cs].reshape(KO, P, HC_N, P).transpose(2, 1, 0, 3)
        wz_shards.append(np.ascontiguousarray(wzs))
        wxns = w_xn[:, cs].reshape(KOX, P, HC_N, P).transpose(2, 1, 0, 3)
        wxn_shards.append(np.ascontiguousarray(wxns))
        # r gate + hn part, fp8 DoubleRow layout [HC_N, P, KO8+KO8H, 2, P]
        w8 = np.empty((HC_N, P, KO8 + KO8H, 2, P), fp8np)
        w8r_ = (wcat_r[:, cs] * WS).astype(fp8np)
        w8[:, :, :KO8] = w8r_.reshape(KO8, 2, P, HC_N, P).transpose(3, 2, 0, 1, 4)
        w8h_ = (w_hn[:, cs] * WS).astype(fp8np)
        w8[:, :, KO8:] = w8h_.reshape(KO8H, 2, P, HC_N, P).transpose(3, 2, 0, 1, 4)
        w8_shards.append(np.ascontiguousarray(w8))
        if with_bias:
            # [4, H_CORE] -> [4, HC_N, P] -> [P, 4, HC_N]
            bs = biases[:, cs].reshape(4, HC_N, P).transpose(2, 0, 1)
            bias_shards.append(np.ascontiguousarray(bs.astype(np.float32)))

    # per batch-shard: lhsT blocks [NBLK, P, KO, MBLK], fp8 copy, hT slices
    lhsT_shards = []
    lhsT8_shards = []
    hT_shards = []
    for bi in range(B_SHARDS):
        sl = slice(bi * B_CORE, (bi + 1) * B_CORE)
        lhsT_full = np.empty((K, B_CORE), np.float16)
        lhsT_full[:F] = x[sl].T
        lhsT_full[F:] = h[sl].T
        # [K, B_CORE] -> [KO, P, NBLK, MBLK] -> [NBLK, P, KO, MBLK]
        lt = lhsT_full.reshape(KO, P, NBLK, MBLK).transpose(2, 1, 0, 3)
        lhsT_shards.append(np.ascontiguousarray(lt))
        l8 = np.empty((K, B_CORE), fp8np)
        l8[:F] = x[sl].T.astype(fp8np)
        l8[F:] = h[sl].T.astype(fp8np)
        # [K, B_CORE] -> [KO8, 2, P, NBLK, MBLK] -> [NBLK, P, KO8, 2, MBLK]
        l8 = l8.reshape(KO8, 2, P, NBLK, MBLK).transpose(3, 2, 0, 1, 4)
        lhsT8_shards.append(np.ascontiguousarray(l8))
        hT_shards.append(np.ascontiguousarray(h[sl].T.astype(np.float16)))  # [H, B_CORE]

    in_maps = []
    for c in range(N_CORES):
        bi, hj = divmod(c, H_SHARDS)
        m = {
            "lhsT": lhsT_shards[bi],
            "lhsT8": lhsT8_shards[bi],
            "wz": wz_shards[hj],
            "wxn": wxn_shards[hj],
            "w8rh": w8_shards[hj],
            "hT": np.ascontiguousarray(
                hT_shards[bi][hj * H_CORE : (hj + 1) * H_CORE]
            ),
        }
        if with_bias:
            m["bias"] = bias_shards[hj]
        in_maps.append(m)
    return in_maps, with_bias


def kernel(h, x, W_ir, W_iz, W_in, b_ir, b_iz, b_in, W_hr, W_hz, W_hn, b_hn):
    in_maps, with_bias = prepare_in_maps(
        h, x, W_ir, W_iz, W_in, b_ir, b_iz, b_in, W_hr, W_hz, W_hn, b_hn
    )
    nc = get_program(with_bias)
    res = run_bass_kernel_spmd(nc, in_maps, list(range(N_CORES)))
    new_h = np.empty((B, H), np.float32)
    for c in range(N_CORES):
        bi, hj = divmod(c, H_SHARDS)
        outT = res.results[c]["out"]  # [H_CORE, B_CORE]
        new_h[bi * B_CORE : (bi + 1) * B_CORE, hj * H_CORE : (hj + 1) * H_CORE] = (
            outT.T
        )
    return (new_h, new_h)


# revision 19
# speedup vs baseline: 1.1909x; 1.1909x over previous
"""Trainium2 Bass kernel for a fused GRUCell step.

Math (reference):
    xi = x @ [W_ir W_iz W_in] + [b_ir b_iz b_in]
    hh = h @ [W_hr W_hz W_hn]
    r = sigmoid(xr + hr); z = sigmoid(xz + hz)
    n = tanh(xn + r * (hn + b_hn))
    new_h = (1 - z) * n + z * h

Strategy (v3, HW-measured 281us vs 397us v1 baseline; rel err 1.21e-2):

- 2D shard — batch 4-way x hidden 2-way (8 cores), each core a
  [B_CORE=4096, H_CORE=512] output tile. Weights per core shrink to ~4.5MB
  (vs 12.6MB replicated), eliminating the startup DMA wall that dominated
  v1 (first matmul at t=52us waiting for all replicated weights to land).
- The GEMM runs with the *weights* stationary and 512-wide batch blocks
  moving, so the weight-arrival rate needed at startup is 8x lower than
  batch-stationary. Outputs land transposed ([H-partition, batch-free]);
  h is supplied host-transposed and the final unshard transposes back.
  Per-gate biases become per-partition scalars, fused into ScalarE
  activations for free.
- Precision: z gate and the x-part of n stay fp16 (1 cycle/row on the PE);
  the r gate and the h-part of n run fp8(e4m3) with perf_mode=DoubleRow,
  packing K=256 per matmul — 2x PE throughput for those sweeps. Weights are
  pre-scaled by WS=16 into the e4m3 normal range; the 1/WS rescale folds
  into the sigmoid activation scale (r) / one tensor_scalar_mul (hn).
  Measured end-to-end rel err 1.21e-2 (numpy-simulated and HW-confirmed to
  4 digits) against the 2e-2 gate; all-fp8 (3e-2) and z-fp8 (1.8e-2 alone)
  do not fit. The hn DR sweep ends each chunk so it sits adjacent to the
  next chunk's DR r-sweep — fp16<->fp8 PE mode flips cost ~190ns each and
  are kept to 2 per chunk.
- DMA issue order is hand-scheduled on the sync queue (one FIFO, ~290GB/s
  aggregate across 16 engines, bulk flow starts ~10.5us into the kernel):
  first-chunk weights and the first lhsT block go first, remaining weights
  stream in consumption order during block 0, lhsT blocks prefetch mid-block.
  First matmul issues ~13us in; the matmul stream then runs at the 216ns/MM
  issue floor with ~15us total of residual stalls (block-1 lhsT arrival is
  DMA-bandwidth-bound, plus ~200ns DoubleRow LDWEIGHTS bubbles: a DR
  weight load is 256 columns ~ 213ns, nearly the full 216ns matmul slot).
- The last chunk's epilogue runs in two 256-column pieces to shorten the
  post-matmul tail (the gate chain is ~5us deep on DVE/ScalarE otherwise).
  Keep the phn rescale on the vector engine: moving it to ScalarE
  serializes the epilogue chain and cost +46us when tried.

Accumulation is fp32 in PSUM throughout: pr/pz/pxn/phn each take a full
PSUM bank, double-buffered = all 8 banks.
"""

import os
import sys

import numpy as np

sys.path.insert(0, "/opt/trn_rl_repo")
os.environ.setdefault("MYCRO_LOCAL_CACHE", "1")

import concourse.bass as bass  # noqa: E402
import concourse.mybir as mybir  # noqa: E402
import concourse.tile as tile  # noqa: E402
from concourse import bacc  # noqa: E402
from concourse.bass_utils import run_bass_kernel_spmd  # noqa: E402

N_CORES = 8
B_SHARDS = 4
H_SHARDS = 2
B = 16384
F = 1024  # input feature dim
H = 1024  # hidden dim
K = F + H  # GEMM contraction dim (x features then h features)
P = 128
KO = K // P  # 16 k-chunks of 128
KOX = F // P  # 8 k-chunks belonging to the x part
B_CORE = B // B_SHARDS  # 4096
H_CORE = H // H_SHARDS  # 512
MBLK = 512  # batch columns per moving-operand block (= PSUM bank width)
NBLK = B_CORE // MBLK  # 8
HC_N = H_CORE // P  # 4 hidden-column chunks of 128 (PSUM partition dim)
KO8 = K // 256  # 8 k-chunks of 256 for fp8 DoubleRow (r gate)
KO8X = F // 256  # 4 fp8 k-chunks belonging to the x part
KO8H = H // 256  # 4 fp8 k-chunks for the hn part
WS = 16.0  # fp8 weight scale: lifts w std ~0.031 into e4m3 normal range


def build_gru_program(with_bias: bool) -> bass.Bass:
    """One SPMD program; every core runs it on its own (batch, hidden) tile."""
    fp16 = mybir.dt.float16
    f32 = mybir.dt.float32

    # Bacc (not plain Bass): its compile pipeline splits multi-sem waits into
    # event semaphores — walrus rejects >1 wait on most engine instructions.
    nc = bacc.Bacc()
    # Host-prearranged so every DMA lands contiguously per partition:
    # lhsT[b, p, ko, m] = concat(x,h).T[ko*P+p, b*MBLK+m]
    lhsT = nc.declare_dram_parameter("lhsT", [NBLK, P, KO, MBLK], fp16, isOutput=False)
    fp8 = mybir.dt.float8e4
    # z gate (full K) and xn part (first F rows of n gate) stay fp16
    wz = nc.declare_dram_parameter("wz", [HC_N, P, KO, P], fp16, isOutput=False)
    wxn = nc.declare_dram_parameter("wxn", [HC_N, P, KOX, P], fp16, isOutput=False)
    # r gate (full K) + hn part in fp8 DoubleRow layout, combined:
    # j in [0,KO8): WS*Wcat_r[j*256+i*128+p, hc*P+n]
    # j in [KO8, KO8+KO8H): WS*W_hn[(j-KO8)*256+i*128+p, hc*P+n]
    w8rh = nc.declare_dram_parameter(
        "w8rh", [HC_N, P, KO8 + KO8H, 2, P], fp8, isOutput=False
    )
    # fp8 copy of the moving operand for the r gate, DoubleRow-paired
    lhsT8 = nc.declare_dram_parameter(
        "lhsT8", [NBLK, P, KO8, 2, MBLK], fp8, isOutput=False
    )
    # hT[j, m] = h_shard[m, j]  (host-transposed h slice; fp16 — only feeds
    # the z*h blend, where fp16 rounding of h is ~3e-4 relative)
    hT = nc.declare_dram_parameter("hT", [H_CORE, B_CORE], fp16, isOutput=False)
    if with_bias:
        # bias[p, g, hc] = b_g[hj*H_CORE + hc*P + p]; g: 0=b_ir 1=b_iz 2=b_in 3=b_hn
        biasp = nc.declare_dram_parameter("bias", [P, 4, HC_N], f32, isOutput=False)
    out = nc.declare_dram_parameter("out", [H_CORE, B_CORE], f32, isOutput=True)

    Sigmoid = mybir.ActivationFunctionType.Sigmoid
    Tanh = mybir.ActivationFunctionType.Tanh

    with tile.TileContext(nc) as tc:
        with (
            tc.tile_pool(name="wpool", bufs=1) as wpool,
            tc.tile_pool(name="lpool", bufs=3) as lpool,
            tc.tile_pool(name="hpool", bufs=2) as hpool,
            tc.tile_pool(name="opool", bufs=3) as opool,
            tc.tile_pool(name="epool", bufs=2) as epool,
            tc.tile_pool(name="psum", bufs=2, space="PSUM") as psum,
        ):
            wsb = {}
            w8sb = {}

            def load_wz(hc: int):
                t = wpool.tile([P, KO, P], fp16, tag=f"wz_{hc}")
                nc.sync.dma_start(t[:], wz[hc])
                wsb[("z", hc)] = t

            def load_wxn(hc: int):
                t = wpool.tile([P, KOX, P], fp16, tag=f"wxn_{hc}")
                nc.sync.dma_start(t[:], wxn[hc])
                wsb[("xn", hc)] = t

            def load_w8(hc: int):
                t = wpool.tile([P, KO8 + KO8H, 2, P], fp8, tag=f"w8_{hc}")
                nc.sync.dma_start(t[:], w8rh[hc])
                w8sb[hc] = t

            def load_lt8(b: int):
                t = lpool.tile([P, KO8, 2, MBLK], fp8, tag="lt8")
                half = KO8 // 2
                nc.sync.dma_start(t[:, 0:half, :, :], lhsT8[b, :, 0:half, :, :])
                nc.sync.dma_start(t[:, half:KO8, :, :], lhsT8[b, :, half:KO8, :, :])
                return t

            def load_lt(b: int):
                # two ko-halves so the first matmuls start after 1MB, not 2MB
                t = lpool.tile([P, KO, MBLK], fp16, tag="lt")
                half = KO // 2
                nc.sync.dma_start(t[:, 0:half, :], lhsT[b, :, 0:half, :])
                nc.sync.dma_start(t[:, half:KO, :], lhsT[b, :, half:KO, :])
                return t

            # --- startup-critical DMA order (sync queue is one FIFO) ---
            load_w8(0)
            lt8 = load_lt8(0)
            load_wz(0)
            lt = load_lt(0)
            load_wxn(0)
            bias_sb = None
            if with_bias:
                bias_sb = wpool.tile([P, 4, HC_N], f32, tag="bias_sb")
                nc.sync.dma_start(bias_sb[:], biasp[:])

            ht_tiles = {}

            def load_ht(b: int, hc: int):
                t = hpool.tile([P, MBLK], fp16, tag=f"ht{hc}")
                nc.sync.dma_start(
                    t[:], hT[hc * P : (hc + 1) * P, b * MBLK : (b + 1) * MBLK]
                )
                ht_tiles[(b, hc)] = t

            load_ht(0, 0)

            # PE warmup: ~100 tiny matmuls on a zeroed scratch tile while the
            # first operands stream in (~13us). Keeps the PE HAM activity
            # window busy so it unthrottles from 1.2GHz to 2.4GHz before the
            # real matmul stream starts (saves ~14 cold matmuls at 2x cost).
            warm_w = wpool.tile([P, P], fp16, tag="warm_w")
            nc.vector.memset(warm_w[:], 0.0)
            # allocate all four psum tags so the ring phases stay aligned —
            # rotating only pr puts it anti-phase with pz/pxn/phn and the
            # per-chunk bank handoff then serializes against the epilogue
            warm_ps = psum.tile([P, MBLK], f32, tag="pr")
            warm_pz = psum.tile([P, MBLK], f32, tag="pz")  # noqa: F841
            warm_pxn = psum.tile([P, MBLK], f32, tag="pxn")  # noqa: F841
            warm_phn = psum.tile([P, MBLK], f32, tag="phn")  # noqa: F841
            for _ in range(100):
                nc.tensor.matmul(
                    warm_ps[:, 0:P], warm_w[:], warm_w[:], start=True, stop=True
                )

            lt_next = None
            lt8_next = None
            for b in range(NBLK):
                # snake order: odd blocks walk hc in reverse, so the first
                # chunk of block 1 needs the weights that arrive last (hc=3)
                # just when they land, instead of stalling on them mid-block-0
                hcs = range(HC_N) if b % 2 == 0 else range(HC_N - 1, -1, -1)
                for ci, hc in enumerate(hcs):
                    # block 0 pulls in the remaining weights one chunk ahead
                    # of use (incl. hc=3 before the block-1 lhsT prefetch)
                    if b == 0 and ci < HC_N - 1:
                        load_w8(ci + 1)
                        load_wz(ci + 1)
                        load_wxn(ci + 1)
                        # block 0's h tiles, early: their epilogues must not
                        # lag more than the PSUM double-buffer allows
                        load_ht(0, ci + 1)
                        if ci == 1:
                            load_ht(1, 3)  # block 1 starts at hc=3 (snake)
                    # prefetch next batch block mid-way through this one
                    if ci == 2 and b + 1 < NBLK:
                        lt8_next = load_lt8(b + 1)
                        lt_next = load_lt(b + 1)

                    if (b, hc) in ht_tiles:
                        ht = ht_tiles.pop((b, hc))
                    else:
                        ht = hpool.tile([P, MBLK], fp16, tag=f"ht{hc}")
                        nc.sync.dma_start(
                            ht[:],
                            hT[hc * P : (hc + 1) * P, b * MBLK : (b + 1) * MBLK],
                        )

                    pr = psum.tile([P, MBLK], f32, tag="pr")
                    pz = psum.tile([P, MBLK], f32, tag="pz")
                    pxn = psum.tile([P, MBLK], f32, tag="pxn")
                    phn = psum.tile([P, MBLK], f32, tag="phn")

                    # gate sweeps: stationary = weight chunk, moving = batch
                    # r gate: fp8 DoubleRow, K=256 per matmul, result is WS*(xr+hr)
                    for ko8 in range(KO8):
                        nc.tensor.matmul(
                            pr[:],
                            w8sb[hc][:, ko8, :, :],
                            lt8[:, ko8, :, :],
                            start=(ko8 == 0),
                            stop=(ko8 == KO8 - 1),
                            perf_mode=mybir.MatmulPerfMode.DoubleRow,
                        )
                    for ko in range(KO):
                        nc.tensor.matmul(
                            pz[:],
                            wsb[("z", hc)][:, ko, :],
                            lt[:, ko, :],
                            start=(ko == 0),
                            stop=(ko == KO - 1),
                        )
                    for ko in range(KOX):
                        nc.tensor.matmul(
                            pxn[:],
                            wsb[("xn", hc)][:, ko, :],
                            lt[:, ko, :],
                            start=(ko == 0),
                            stop=(ko == KOX - 1),
                        )
                    # hn part: fp8 DoubleRow (ends the chunk so it sits next
                    # to the following chunk's DR r-sweep — fewer mode flips)
                    for j in range(KO8H):
                        nc.tensor.matmul(
                            phn[:],
                            w8sb[hc][:, KO8 + j, :, :],
                            lt8[:, KO8X + j, :, :],
                            start=(j == 0),
                            stop=(j == KO8H - 1),
                            perf_mode=mybir.MatmulPerfMode.DoubleRow,
                        )

                    sr = epool.tile([P, MBLK], f32, tag="sr")
                    sz = epool.tile([P, MBLK], f32, tag="sz")
                    sn = epool.tile([P, MBLK], f32, tag="sn")
                    tt = epool.tile([P, MBLK], f32, tag="tt")
                    ot = opool.tile([P, MBLK], f32, tag="ot")

                    def epilogue(lo: int, hi: int):
                        s = slice(lo, hi)
                        if with_bias:
                            nc.scalar.activation(
                                sr[:, s],
                                pr[:, s],
                                Sigmoid,
                                bias=bias_sb[:, 0, hc : hc + 1],
                                scale=1.0 / WS,
                            )
                            nc.scalar.activation(
                                sz[:, s],
                                pz[:, s],
                                Sigmoid,
                                bias=bias_sb[:, 1, hc : hc + 1],
                            )
                            nc.vector.tensor_scalar(
                                tt[:, s],
                                phn[:, s],
                                1.0 / WS,
                                bias_sb[:, 3, hc : hc + 1],
                                mybir.AluOpType.mult,
                                mybir.AluOpType.add,
                            )
                            nc.vector.tensor_mul(tt[:, s], sr[:, s], tt[:, s])
                            nc.vector.tensor_add(tt[:, s], tt[:, s], pxn[:, s])
                            nc.scalar.activation(
                                sn[:, s],
                                tt[:, s],
                                Tanh,
                                bias=bias_sb[:, 2, hc : hc + 1],
                            )
                        else:
                            nc.scalar.activation(
                                sr[:, s], pr[:, s], Sigmoid, scale=1.0 / WS
                            )
                            nc.scalar.activation(sz[:, s], pz[:, s], Sigmoid)
                            nc.vector.tensor_scalar_mul(tt[:, s], phn[:, s], 1.0 / WS)
                            nc.vector.tensor_mul(tt[:, s], sr[:, s], tt[:, s])
                            nc.vector.tensor_add(tt[:, s], tt[:, s], pxn[:, s])
                            nc.scalar.activation(sn[:, s], tt[:, s], Tanh)
                        nc.vector.tensor_sub(tt[:, s], ht[:, s], sn[:, s])
                        nc.vector.tensor_mul(tt[:, s], tt[:, s], sz[:, s])
                        nc.vector.tensor_add(ot[:, s], sn[:, s], tt[:, s])
                        nc.sync.dma_start(
                            out[
                                hc * P : (hc + 1) * P,
                                b * MBLK + lo : b * MBLK + hi,
                            ],
                            ot[:, s],
                        )

                    if b == NBLK - 1 and ci == HC_N - 1:
                        # last chunk: pipeline the epilogue in column pieces so
                        # the post-matmul tail is short
                        for lo in range(0, MBLK, 2 * P):
                            epilogue(lo, lo + 2 * P)
                    else:
                        epilogue(0, MBLK)
                if lt_next is not None:
                    lt = lt_next
                    lt8 = lt8_next
                    lt_next = None
    nc.finalize()
    return nc


_PROGRAM_CACHE: dict = {}


def get_program(with_bias: bool) -> bass.Bass:
    if with_bias not in _PROGRAM_CACHE:
        _PROGRAM_CACHE[with_bias] = build_gru_program(with_bias)
    return _PROGRAM_CACHE[with_bias]


def prepare_in_maps(h, x, W_ir, W_iz, W_in, b_ir, b_iz, b_in, W_hr, W_hz, W_hn, b_hn):
    """Host-side shard + layout prep. Returns (in_maps, with_bias)."""
    h = np.ascontiguousarray(np.asarray(h, dtype=np.float32))
    x = np.ascontiguousarray(np.asarray(x, dtype=np.float32))
    assert x.shape == (B, F) and h.shape == (B, H), (x.shape, h.shape)

    import ml_dtypes

    fp8np = ml_dtypes.float8_e4m3
    wcat_z = np.concatenate([W_iz, W_hz], axis=0).astype(np.float16)  # [K, H]
    w_xn = np.asarray(W_in, np.float32).astype(np.float16)  # [F, H]
    wcat_r = np.concatenate([W_ir, W_hr], axis=0).astype(np.float32)  # [K, H]
    w_hn = np.asarray(W_hn, np.float32)  # [H, H]

    br = np.asarray(b_ir, np.float32)
    bz = np.asarray(b_iz, np.float32)
    bn = np.asarray(b_in, np.float32)
    bhn = np.asarray(b_hn, np.float32)
    biases = np.stack([br, bz, bn, bhn])  # [4, H]
    with_bias = bool(np.any(biases != 0.0))

    # per H-shard: weights in the exact SBUF layout
    wz_shards = []
    wxn_shards = []
    w8_shards = []
    bias_shards = []
    for hj in range(H_SHARDS):
        cs = slice(hj * H_CORE, (hj + 1) * H_CORE)
        # [K, H_CORE] -> [KO, P, HC_N, P] -> [HC_N, P, KO, P]
        wzs = wcat_z[:, cs].reshape(KO, P, HC_N, P).transpose(2, 1, 0, 3)
        wz_shards.append(np.ascontiguousarray(wzs))
        wxns = w_xn[:, cs].reshape(KOX, P, HC_N, P).transpose(2, 1, 0, 3)
        wxn_shards.append(np.ascontiguousarray(wxns))
        # r gate + hn part, fp8 DoubleRow layout [HC_N, P, KO8+KO8H, 2, P]
        w8 = np.empty((HC_N, P, KO8 + KO8H, 2, P), fp8np)
        w8r_ = (wcat_r[:, cs] * WS).astype(fp8np)
        w8[:, :, :KO8] = w8r_.reshape(KO8, 2, P, HC_N, P).transpose(3, 2, 0, 1, 4)
        w8h_ = (w_hn[:, cs] * WS).astype(fp8np)
        w8[:, :, KO8:] = w8h_.reshape(KO8H, 2, P, HC_N, P).transpose(3, 2, 0, 1, 4)
        w8_shards.append(np.ascontiguousarray(w8))
        if with_bias:
            # [4, H_CORE] -> [4, HC_N, P] -> [P, 4, HC_N]
            bs = biases[:, cs].reshape(4, HC_N, P).transpose(2, 0, 1)
            bias_shards.append(np.ascontiguousarray(bs.astype(np.float32)))

    # per batch-shard: lhsT blocks [NBLK, P, KO, MBLK], fp8 copy, hT slices
    lhsT_shards = []
    lhsT8_shards = []
    hT_shards = []
    for bi in range(B_SHARDS):
        sl = slice(bi * B_CORE, (bi + 1) * B_CORE)
        lhsT_full = np.empty((K, B_CORE), np.float16)
        lhsT_full[:F] = x[sl].T
        lhsT_full[F:] = h[sl].T
        # [K, B_CORE] -> [KO, P, NBLK, MBLK] -> [NBLK, P, KO, MBLK]
        lt = lhsT_full.reshape(KO, P, NBLK, MBLK).transpose(2, 1, 0, 3)
        lhsT_shards.append(np.ascontiguousarray(lt))
        l8 = np.empty((K, B_CORE), fp8np)
        l8[:F] = x[sl].T.astype(fp8np)
        l8[F:] = h[sl].T.astype(fp8np)
        # [K, B_CORE] -> [KO8, 2, P, NBLK, MBLK] -> [NBLK, P, KO8, 2, MBLK]
        l8 = l8.reshape(KO8, 2, P, NBLK, MBLK).transpose(3, 2, 0, 1, 4)
        lhsT8_shards.append(np.ascontiguousarray(l8))
        hT_shards.append(np.ascontiguousarray(h[sl].T.astype(np.float16)))  # [H, B_CORE]

    in_maps = []
    for c in range(N_CORES):
        bi, hj = divmod(c, H_SHARDS)
        m = {
            "lhsT": lhsT_shards[bi],
            "lhsT8": lhsT8_shards[bi],
            "wz": wz_shards[hj],
            "wxn": wxn_shards[hj],
            "w8rh": w8_shards[hj],
            "hT": np.ascontiguousarray(
                hT_shards[bi][hj * H_CORE : (hj + 1) * H_CORE]
            ),
        }
        if with_bias:
            m["bias"] = bias_shards[hj]
        in_maps.append(m)
    return in_maps, with_bias


def kernel(h, x, W_ir, W_iz, W_in, b_ir, b_iz, b_in, W_hr, W_hz, W_hn, b_hn):
    in_maps, with_bias = prepare_in_maps(
        h, x, W_ir, W_iz, W_in, b_ir, b_iz, b_in, W_hr, W_hz, W_hn, b_hn
    )
    nc = get_program(with_bias)
    res = run_bass_kernel_spmd(nc, in_maps, list(range(N_CORES)))
    new_h = np.empty((B, H), np.float32)
    for c in range(N_CORES):
        bi, hj = divmod(c, H_SHARDS)
        outT = res.results[c]["out"]  # [H_CORE, B_CORE]
        new_h[bi * B_CORE : (bi + 1) * B_CORE, hj * H_CORE : (hj + 1) * H_CORE] = (
            outT.T
        )
    return (new_h, new_h)
